# revision 11
# baseline (speedup 1.0000x reference)
"""CLUBMean loss kernel for Trainium2, 8-core data-parallel.

Math: with x_vec = mean_s(x), y_vec = mean_s(y), mu = MLP(x_vec):
  positive_i = -||mu_i - y_i||^2 / 2
  negative_i = -mean_j ||y_j - mu_i||^2 / 2
             = -(S2/N - 2 mu_i . Ey + ||mu_i||^2) / 2      (exact expansion)
  loss = mean_i(positive_i - negative_i)

Each core handles 128 of the 1024 samples and emits partial sums:
  out_vec (128,4): columns [Ey lo, Ey hi, Mu lo, Mu hi] summed over its samples
  out_row (1,2,3,128): per-sample ||mu-y||^2, ||mu||^2, ||y||^2 (split by D-half)
The host all-reduces the partials in float64 and applies the closed form.

Streaming layout: 24 chunks of 32 channels (1 MiB each), alternating between
the two HWDGE queues (sync = even chunks, scalar = odd chunks + weights).
DVE pools each chunk (spatial sum); PE transposes pooled vectors to
channel-major; the MLP runs as fp32 PE matmuls accumulated in PSUM while the
stream continues; the epilogue is split per D-half so only the second half
sits in the tail.
"""

import sys

sys.path.insert(0, "/opt/trn_rl_repo")

from contextlib import ExitStack

import numpy as np

import concourse.bass as bass
import concourse.mybir as mybir
from concourse.bass_utils import run_bass_kernel_spmd
from concourse.masks import make_identity

N = 1024
P = 128            # samples per core
XC, YC, HID, S = 512, 256, 512, 64
CH = 32            # channel chunk per streamed DMA (1 MiB)
NX = XC // CH      # 16 x chunks
NY = YC // CH      # 8 y chunks
NCHUNK = NX + NY   # 24
NBUF = 12          # stream buffer ring
NXV = 4            # pooled-vector ring
F32 = mybir.dt.float32
AX = mybir.AxisListType
ALU = mybir.AluOpType
ACTF = mybir.ActivationFunctionType

_CACHE = {}


def build_nc(debug=False):
    nc = bass.Bass()
    x = nc.dram_tensor("x", [P, XC, S], F32, kind="ExternalInput")
    y = nc.dram_tensor("y", [P, YC, S], F32, kind="ExternalInput")
    w1 = nc.dram_tensor("w1", [XC, HID], F32, kind="ExternalInput")
    b1 = nc.dram_tensor("b1", [P, 4], F32, kind="ExternalInput")
    w2 = nc.dram_tensor("w2", [HID, YC], F32, kind="ExternalInput")
    b2 = nc.dram_tensor("b2", [P, 2], F32, kind="ExternalInput")
    out_vec = nc.dram_tensor("out_vec", [P, 4], F32, kind="ExternalOutput")
    out_row = nc.dram_tensor("out_row", [1, 2, 3, P], F32, kind="ExternalOutput")
    if debug:
        dbg_xvT = nc.dram_tensor("dbg_xvT", [P, 4, P], F32, kind="ExternalOutput")
        dbg_hT = nc.dram_tensor("dbg_hT", [P, 4, P], F32, kind="ExternalOutput")
        dbg_muT = nc.dram_tensor("dbg_muT", [P, 2, P], F32, kind="ExternalOutput")

    ctx = ExitStack()
    with ctx:
        sb = lambda name, shape: ctx.enter_context(nc.sbuf_tensor(name, shape, F32))
        ps = lambda name, shape: ctx.enter_context(nc.psum_tensor(name, shape, F32))
        sem = lambda name: ctx.enter_context(nc.semaphore(name))

        xbuf = [sb(f"xbuf{i}", [P, CH, S]) for i in range(NBUF)]
        xv = [sb(f"xv{i}", [P, CH]) for i in range(NXV)]
        xvT = sb("xvT", [P, 4, P])
        yvT = sb("yvT", [P, 2, P])
        hT = sb("hT", [P, 4, P])
        muT = sb("muT", [P, 2, P])
        dtmp = sb("dtmp", [P, P])
        sq_all = sb("sq_all", [P, 2, 3, P])   # [:, m2, 0..2, :] = sqd, sqmu, sqy
        w1sb = sb("w1sb", [P, 4, HID])
        w2sb = sb("w2sb", [P, 4, YC])
        b1sb = sb("b1sb", [P, 4])
        b2sb = sb("b2sb", [P, 2])
        ident = sb("ident", [P, P])
        ones = sb("ones", [P, 1])
        stat = sb("stat", [P, 4])
        rows = sb("rows", [1, 2, 3, P])

        pt = [ps(f"pt{i}", [CH, P]) for i in range(2)]
        ph = ps("ph", [P, 4, P])
        pmu = ps("pmu", [P, 2, P])
        prow = [ps(f"prow{i}", [1, 3, P]) for i in range(2)]

        # one sem per chunk: a DMA's +16 arrives as +1 from each of the 16
        # DGE lanes, so cumulative thresholds across chunks are unsound
        dch = [sem(f"dch{i}") for i in range(NCHUNK)]
        dw = sem("dw")              # weight DMAs (4 x 16 = 64)
        dout = sem("dout")
        s_const = sem("s_const")
        s_pool = sem("s_pool")
        s_tp = sem("s_tp")
        s_cp = sem("s_cp")
        s_hmm = sem("s_hmm")
        s_relu = sem("s_relu")
        s_mumm = sem("s_mumm")
        s_mubias = sem("s_mubias")
        s_sq = [sem("s_sq0"), sem("s_sq1")]
        s_stat = sem("s_stat")
        s_row = [sem("s_row0"), sem("s_row1")]
        s_rowscp = sem("s_rowscp")

        def chunk_src(i):
            if i < NX:
                return x[:, i * CH:(i + 1) * CH, :]
            j = i - NX
            return y[:, j * CH:(j + 1) * CH, :]

        def chunk_done(e, i):
            e.wait_ge(dch[i], 16)

        def issue_chunk(e, i):
            e.dma_start(out=xbuf[i % NBUF][:, :, :], in_=chunk_src(i)).then_inc(
                dch[i], 16
            )

        def _half_epilogue(e, m):
            e.wait_ge(s_mubias, m + 1)
            e.wait_ge(s_cp, NX + 4 * (m + 1))
            e.tensor_sub(dtmp[:, :], muT[:, m, :], yvT[:, m, :])
            e.tensor_mul(sq_all[:, m, 0, :], dtmp[:, :], dtmp[:, :])
            e.tensor_mul(sq_all[:, m, 1, :], muT[:, m, :], muT[:, m, :])
            e.tensor_mul(
                sq_all[:, m, 2, :], yvT[:, m, :], yvT[:, m, :]
            ).then_inc(s_sq[m], 1)
            e.tensor_reduce(
                stat[:, m:m + 1], yvT[:, m, :], axis=AX.X, op=ALU.add
            )
            inst = e.tensor_reduce(
                stat[:, 2 + m:3 + m], muT[:, m, :], axis=AX.X, op=ALU.add
            )
            if m == 1:
                inst.then_inc(s_stat, 1)

        with nc.Block() as block:

            @block.sync
            def _(e):
                for i in range(0, NBUF, 2):       # 0,2,4,6,8,10 upfront
                    issue_chunk(e, i)
                for i in range(NBUF, NCHUNK, 2):  # ring-guarded
                    e.wait_ge(s_pool, i - NBUF + 1)
                    issue_chunk(e, i)
                e.wait_ge(s_stat, 1)
                e.dma_start(out=out_vec[:, :], in_=stat[:, :]).then_inc(dout, 16)
                e.wait_ge(s_rowscp, 1)
                e.dma_start(out=out_row[:, :, :, :], in_=rows[:, :, :, :]).then_inc(
                    dout, 16
                )
                if debug:
                    e.dma_start(out=dbg_xvT[:, :, :], in_=xvT[:, :, :]).then_inc(dout, 16)
                    e.dma_start(out=dbg_hT[:, :, :], in_=hT[:, :, :]).then_inc(dout, 16)
                    e.dma_start(out=dbg_muT[:, :, :], in_=muT[:, :, :]).then_inc(dout, 16)
                e.wait_ge(dout, 32 + (48 if debug else 0))

            @block.gpsimd
            def _(e):
                make_identity(nc, ident[:, :])
                e.memset(ones[:, :], 1.0).then_inc(s_const, 1)

            @block.vector
            def _(e):
                for i in range(NCHUNK):
                    chunk_done(e, i)
                    if i >= NXV:
                        e.wait_ge(s_tp, i - NXV + 1)
                    e.tensor_reduce(
                        xv[i % NXV][:, :],
                        xbuf[i % NBUF][:, :, :],
                        axis=AX.X,
                        op=ALU.add,
                    ).then_inc(s_pool, 1)
                    if i == NCHUNK - 5:
                        # first D-half epilogue: y chunks 16..19 are pooled,
                        # transposed and copied; mu has been ready for a while
                        _half_epilogue(e, 0)
                _half_epilogue(e, 1)

            @block.tensor
            def _(e):
                e.wait_ge(s_const, 1)
                for i in range(NCHUNK):
                    e.wait_ge(s_pool, i + 1)
                    if i >= 2:
                        e.wait_ge(s_cp, i - 1)
                    e.transpose(
                        pt[i % 2][:, :], xv[i % NXV][:, :], ident[:, :]
                    ).then_inc(s_tp, 1)
                    if i == NX - 1:
                        e.wait_ge(s_cp, NX)
                        e.wait_ge(dw, 64)
                        for m in range(4):
                            for k in range(4):
                                mm = e.matmul(
                                    ph[:, m, :],
                                    w1sb[:, k, m * P:(m + 1) * P],
                                    xvT[:, k, :],
                                    start=(k == 0),
                                    stop=(k == 3),
                                )
                        mm.then_inc(s_hmm, 1)
                    if i == NX:
                        e.wait_ge(s_relu, 4)
                        for m in range(2):
                            for k in range(4):
                                mm = e.matmul(
                                    pmu[:, m, :],
                                    w2sb[:, k, m * P:(m + 1) * P],
                                    hT[:, k, :],
                                    start=(k == 0),
                                    stop=(k == 3),
                                )
                        mm.then_inc(s_mumm, 1)
                    if i == NCHUNK - 4:
                        e.wait_ge(s_sq[0], 1)
                        e.matmul(
                            prow[0][:, :, :],
                            ones[:, :],
                            sq_all[:, 0, :, :],
                            start=True,
                            stop=True,
                        ).then_inc(s_row[0], 1)
                e.wait_ge(s_sq[1], 1)
                e.matmul(
                    prow[1][:, :, :],
                    ones[:, :],
                    sq_all[:, 1, :, :],
                    start=True,
                    stop=True,
                ).then_inc(s_row[1], 1)

            @block.scalar
            def _(e):
                # weight loads + odd stream chunks ride the scalar HWDGE queue
                issue_chunk(e, 1)
                issue_chunk(e, 3)
                e.dma_start(
                    out=w1sb[:, :, :],
                    in_=w1[:, :].rearrange("(k p) h -> p k h", p=P),
                ).then_inc(dw, 16)
                e.dma_start(
                    out=w2sb[:, :, :],
                    in_=w2[:, :].rearrange("(k p) h -> p k h", p=P),
                ).then_inc(dw, 16)
                e.dma_start(out=b1sb[:, :], in_=b1[:, :]).then_inc(dw, 16)
                e.dma_start(out=b2sb[:, :], in_=b2[:, :]).then_inc(dw, 16)
                for i in range(5, NBUF, 2):       # 5,7,9,11 upfront
                    issue_chunk(e, i)
                for i in range(NCHUNK):
                    e.wait_ge(s_tp, i + 1)
                    if i < NX:
                        k, q = i // 4, i % 4
                        dst = xvT[q * CH:(q + 1) * CH, k, :]
                    else:
                        j = i - NX
                        k, q = j // 4, j % 4
                        dst = yvT[q * CH:(q + 1) * CH, k, :]
                    # fold the 1/64 spatial mean into the transpose copy (exact)
                    e.activation(
                        dst, pt[i % 2][:CH, :], ACTF.Copy, scale=1.0 / S
                    ).then_inc(s_cp, 1)
                    if i % 2 == 1 and i + NBUF < NCHUNK:
                        # ring-guarded issue of odd chunk i+NBUF; pool(i) is
                        # already implied by copy(i) but wait explicitly
                        e.wait_ge(s_pool, i + 1)
                        issue_chunk(e, i + NBUF)
                    if i == NX - 1:
                        e.wait_ge(s_hmm, 1)
                        for m in range(4):
                            e.activation(
                                hT[:, m, :],
                                ph[:, m, :],
                                ACTF.Relu,
                                bias=b1sb[:, m:m + 1],
                            ).then_inc(s_relu, 1)
                    if i == NX:
                        e.wait_ge(s_mumm, 1)
                        for m in range(2):
                            e.activation(
                                muT[:, m, :],
                                pmu[:, m, :],
                                ACTF.Identity,
                                bias=b2sb[:, m:m + 1],
                            ).then_inc(s_mubias, 1)
                    if i == NCHUNK - 3:
                        e.wait_ge(s_row[0], 1)
                        e.activation(
                            rows[:, 0, :, :], prow[0][:, :, :], ACTF.Copy
                        )
                e.wait_ge(s_row[1], 1)
                e.activation(
                    rows[:, 1, :, :], prow[1][:, :, :], ACTF.Copy
                ).then_inc(s_rowscp, 1)

    return nc


def _get_nc():
    if "nc" not in _CACHE:
        _CACHE["nc"] = build_nc()
    return _CACHE["nc"]


def make_in_maps(x_samples, y_samples, W1, b1, W2, b2):
    xs = np.ascontiguousarray(
        np.asarray(x_samples, np.float32).reshape(N, XC, S)
    )
    ys = np.ascontiguousarray(
        np.asarray(y_samples, np.float32).reshape(N, YC, S)
    )
    w1 = np.ascontiguousarray(np.asarray(W1, np.float32))
    w2 = np.ascontiguousarray(np.asarray(W2, np.float32))
    b1r = np.ascontiguousarray(np.asarray(b1, np.float32).reshape(4, P).T)
    b2r = np.ascontiguousarray(np.asarray(b2, np.float32).reshape(2, P).T)
    in_maps = []
    for c in range(8):
        in_maps.append(
            {
                "x": np.ascontiguousarray(xs[c * P:(c + 1) * P]),
                "y": np.ascontiguousarray(ys[c * P:(c + 1) * P]),
                "w1": w1,
                "b1": b1r,
                "w2": w2,
                "b2": b2r,
            }
        )
    return in_maps


def combine(results):
    A = B = S2 = 0.0
    EyN = np.zeros(YC, np.float64)
    MuN = np.zeros(YC, np.float64)
    for c in range(8):
        vec = results[c]["out_vec"].astype(np.float64)    # (128, 4)
        row = results[c]["out_row"].astype(np.float64)    # (1, 2, 3, 128)
        EyN += np.concatenate([vec[:, 0], vec[:, 1]])
        MuN += np.concatenate([vec[:, 2], vec[:, 3]])
        A += row[0, :, 0, :].sum()
        B += row[0, :, 1, :].sum()
        S2 += row[0, :, 2, :].sum()
    ey = EyN / N
    mu = MuN / N
    loss = -(A / N) / 2.0 + 0.5 * (S2 / N - 2.0 * float(mu @ ey) + B / N)
    return np.float32(loss)


def run(inputs, **kwargs):
    nc = _get_nc()
    in_maps = make_in_maps(**inputs)
    res = run_bass_kernel_spmd(nc, in_maps, core_ids=list(range(8)), **kwargs)
    return combine(res.results), res


def kernel(x_samples, y_samples, W1, b1, W2, b2):
    loss, _ = run(
        dict(
            x_samples=x_samples,
            y_samples=y_samples,
            W1=W1,
            b1=b1,
            W2=W2,
            b2=b2,
        )
    )
    return loss


# revision 16
# speedup vs baseline: 1.1043x; 1.1043x over previous
"""CLUBMean loss kernel for Trainium2, 8-core data-parallel.

Math: with x_vec = mean_s(x), y_vec = mean_s(y), mu = MLP(x_vec):
  positive_i = -||mu_i - y_i||^2 / 2
  negative_i = -mean_j ||y_j - mu_i||^2 / 2
             = -(S2/N - 2 mu_i . Ey + ||mu_i||^2) / 2      (exact expansion)
  loss = mean_i(positive_i - negative_i)

Each core handles 128 of the 1024 samples and emits partial sums:
  out_vec (128,4): columns [Ey lo, Ey hi, Mu lo, Mu hi] summed over its samples
  out_row (1,2,3,128): per-sample ||mu-y||^2, ||mu||^2, ||y||^2 (split by D-half)
The host all-reduces the partials in float64 and applies the closed form.

Single HWDGE (sync) queue streams everything: 4 chunks, then the MLP weights,
then the rest of the channel chunks (the last y chunk split fine to shrink the
tail). DVE pools each chunk; PE transposes pooled vectors to channel-major and
runs the MLP as fp32 matmuls accumulated in PSUM while the stream continues;
epilogue work is split per D-half so only a sliver remains after the last DMA.

Each DMA's +16 semaphore arrives as +1 per DGE lane, so chunk completion uses
one semaphore per chunk (cumulative thresholds across chunks are unsound).
"""

import sys

sys.path.insert(0, "/opt/trn_rl_repo")

from contextlib import ExitStack

import numpy as np

import concourse.bass as bass
import concourse.mybir as mybir
from concourse.bass_utils import run_bass_kernel_spmd
from concourse.masks import make_identity

N = 1024
P = 128            # samples per core
XC, YC, HID, S = 512, 256, 512, 64
CH = 32            # channel chunk per streamed DMA (1 MiB)
NBUF = 12          # stream buffer ring
NXV = 4            # pooled-vector ring
F32 = mybir.dt.float32
AX = mybir.AxisListType
ALU = mybir.AluOpType
ACTF = mybir.ActivationFunctionType

# chunk table: (is_y, c0, w). x: 16 x 32ch; y: 8 x 32ch
# (engine partition offsets must be 32-aligned, so chunks stay 32 wide)
CHUNKS = [(0, c * CH, CH) for c in range(16)]
CHUNKS += [(1, c * CH, CH) for c in range(8)]
NCHUNK = len(CHUNKS)   # 24
NX = 16                # x chunks

_CACHE = {}


def build_nc(debug=False):
    nc = bass.Bass()
    x = nc.dram_tensor("x", [P, XC, S], F32, kind="ExternalInput")
    y = nc.dram_tensor("y", [P, YC, S], F32, kind="ExternalInput")
    # all weights packed host-side into final SBUF layout:
    # [w1 (4k x 512h) | w2 (4k x 256c) | b1 (4) | b2 (2)] per partition
    wpack = nc.dram_tensor("wpack", [P, 3078], F32, kind="ExternalInput")
    out_vec = nc.dram_tensor("out_vec", [P, 4], F32, kind="ExternalOutput")
    out_row = nc.dram_tensor("out_row", [1, 2, 3, P], F32, kind="ExternalOutput")
    if debug:
        dbg_xvT = nc.dram_tensor("dbg_xvT", [P, 4, P], F32, kind="ExternalOutput")
        dbg_hT = nc.dram_tensor("dbg_hT", [P, 4, P], F32, kind="ExternalOutput")
        dbg_muT = nc.dram_tensor("dbg_muT", [P, 2, P], F32, kind="ExternalOutput")

    ctx = ExitStack()
    with ctx:
        sb = lambda name, shape: ctx.enter_context(nc.sbuf_tensor(name, shape, F32))
        ps = lambda name, shape: ctx.enter_context(nc.psum_tensor(name, shape, F32))
        sem = lambda name: ctx.enter_context(nc.semaphore(name))

        xbuf = [sb(f"xbuf{i}", [P, CH, S]) for i in range(NBUF)]
        xv = [sb(f"xv{i}", [P, CH]) for i in range(NXV)]
        xvT = sb("xvT", [P, 4, P])
        yvT = sb("yvT", [P, 2, P])
        hT = sb("hT", [P, 4, P])
        muT = sb("muT", [P, 2, P])
        dtmp = sb("dtmp", [P, P])
        sq_all = sb("sq_all", [P, 2, 3, P])   # [:, m2, 0..2, :] = sqd, sqmu, sqy
        wsb = sb("wsb", [P, 3078])
        ident = sb("ident", [P, P])
        ones = sb("ones", [P, 1])
        stat = sb("stat", [P, 4])
        rows = sb("rows", [1, 2, 3, P])

        pt = [ps(f"pt{i}", [CH, P]) for i in range(2)]
        ph = ps("ph", [P, 4, P])
        pmu = ps("pmu", [P, 2, P])
        prow = [ps(f"prow{i}", [1, 3, P]) for i in range(2)]

        # one sem per chunk: a DMA's +16 arrives as +1 from each of the 16
        # DGE lanes, so cumulative thresholds across chunks are unsound
        dch = [sem(f"dch{i}") for i in range(NCHUNK)]
        dw = sem("dw")              # weight DMAs (4 x 16 = 64)
        dout = sem("dout")
        s_const = sem("s_const")
        s_pool = sem("s_pool")
        s_tp = sem("s_tp")
        s_cp = sem("s_cp")
        s_hmm = sem("s_hmm")
        s_relu = sem("s_relu")
        s_mumm = sem("s_mumm")
        s_mubias = sem("s_mubias")
        s_sq = [sem("s_sq0"), sem("s_sq1")]
        s_stat = sem("s_stat")
        s_row = [sem("s_row0"), sem("s_row1")]
        s_rowcp = [sem("s_rowcp0"), sem("s_rowcp1")]

        def chunk_src(i):
            is_y, c0, w = CHUNKS[i]
            t = y if is_y else x
            return t[:, c0:c0 + w, :]

        def issue_chunk(e, i):
            w = CHUNKS[i][2]
            e.dma_start(
                out=xbuf[i % NBUF][:, :w, :], in_=chunk_src(i)
            ).then_inc(dch[i], 16)

        def copy_dst(i):
            is_y, c0, w = CHUNKS[i]
            t = yvT if is_y else xvT
            return t[c0 % P:c0 % P + w, c0 // P, :]

        # vector helpers -------------------------------------------------
        def _early_mu_block(e):
            # everything that only needs muT: squares + Mu stats
            e.wait_ge(s_mubias, 2)
            e.tensor_mul(sq_all[:, 0, 1, :], muT[:, 0, :], muT[:, 0, :])
            e.tensor_mul(sq_all[:, 1, 1, :], muT[:, 1, :], muT[:, 1, :])
            e.tensor_reduce(stat[:, 2:3], muT[:, 0, :], axis=AX.X, op=ALU.add)
            e.tensor_reduce(stat[:, 3:4], muT[:, 1, :], axis=AX.X, op=ALU.add)

        def _half_epilogue(e, m):
            e.wait_ge(s_mubias, 2)
            e.wait_ge(s_cp, 20 if m == 0 else NCHUNK)
            e.tensor_sub(dtmp[:, :], muT[:, m, :], yvT[:, m, :])
            e.tensor_mul(sq_all[:, m, 0, :], dtmp[:, :], dtmp[:, :])
            e.tensor_mul(
                sq_all[:, m, 2, :], yvT[:, m, :], yvT[:, m, :]
            ).then_inc(s_sq[m], 1)
            inst = e.tensor_reduce(
                stat[:, m:m + 1], yvT[:, m, :], axis=AX.X, op=ALU.add
            )
            if m == 1:
                inst.then_inc(s_stat, 1)

        with nc.Block() as block:

            @block.sync
            def _(e):
                for i in range(4):
                    issue_chunk(e, i)
                e.dma_start(out=wsb[:, :], in_=wpack[:, :]).then_inc(dw, 16)
                for i in range(4, NCHUNK):
                    if i >= NBUF:
                        e.wait_ge(s_pool, i - NBUF + 1)   # ring reuse guard
                    issue_chunk(e, i)
                e.wait_ge(s_row[0], 1)
                e.wait_ge(s_rowcp[0], 1)
                e.dma_start(
                    out=out_row[:, 0, :, :], in_=rows[:, 0, :, :]
                ).then_inc(dout, 16)
                e.wait_ge(s_stat, 1)
                e.dma_start(out=out_vec[:, :], in_=stat[:, :]).then_inc(dout, 16)
                e.wait_ge(s_rowcp[1], 1)
                e.dma_start(
                    out=out_row[:, 1, :, :], in_=rows[:, 1, :, :]
                ).then_inc(dout, 16)
                if debug:
                    e.dma_start(out=dbg_xvT[:, :, :], in_=xvT[:, :, :]).then_inc(dout, 16)
                    e.dma_start(out=dbg_hT[:, :, :], in_=hT[:, :, :]).then_inc(dout, 16)
                    e.dma_start(out=dbg_muT[:, :, :], in_=muT[:, :, :]).then_inc(dout, 16)
                e.wait_ge(dout, 48 + (48 if debug else 0))

            @block.gpsimd
            def _(e):
                make_identity(nc, ident[:, :])
                e.memset(ones[:, :], 1.0).then_inc(s_const, 1)

            @block.vector
            def _(e):
                for i in range(NCHUNK):
                    e.wait_ge(dch[i], 16)
                    if i >= NXV:
                        e.wait_ge(s_tp, i - NXV + 1)
                    w = CHUNKS[i][2]
                    e.tensor_reduce(
                        xv[i % NXV][:, :w],
                        xbuf[i % NBUF][:, :w, :],
                        axis=AX.X,
                        op=ALU.add,
                    ).then_inc(s_pool, 1)
                    if i == 21:
                        _early_mu_block(e)
                    if i == 22:
                        _half_epilogue(e, 0)
                _half_epilogue(e, 1)

            @block.tensor
            def _(e):
                e.wait_ge(s_const, 1)
                for i in range(NCHUNK):
                    e.wait_ge(s_pool, i + 1)
                    if i >= 2:
                        e.wait_ge(s_cp, i - 1)
                    w = CHUNKS[i][2]
                    e.transpose(
                        pt[i % 2][:w, :], xv[i % NXV][:, :w], ident[:, :]
                    ).then_inc(s_tp, 1)
                    if i == NX - 1:
                        # h = x_vec @ W1: fp32 accumulation groups must stay
                        # contiguous (interleaving groups miscompiles)
                        e.wait_ge(s_cp, NX)
                        e.wait_ge(dw, 16)
                        for m in range(4):
                            for k in range(4):
                                mm = e.matmul(
                                    ph[:, m, :],
                                    wsb[:, k * 512 + m * P:
                                        k * 512 + (m + 1) * P],
                                    xvT[:, k, :],
                                    start=(k == 0),
                                    stop=(k == 3),
                                )
                        mm.then_inc(s_hmm, 1)
                    if i == NX:
                        e.wait_ge(s_relu, 4)
                        for m in range(2):
                            for k in range(4):
                                mm = e.matmul(
                                    pmu[:, m, :],
                                    wsb[:, 2048 + k * 256 + m * P:
                                        2048 + k * 256 + (m + 1) * P],
                                    hT[:, k, :],
                                    start=(k == 0),
                                    stop=(k == 3),
                                )
                        mm.then_inc(s_mumm, 1)
                    if i == 23:
                        e.wait_ge(s_sq[0], 1)
                        e.matmul(
                            prow[0][:, :, :],
                            ones[:, :],
                            sq_all[:, 0, :, :],
                            start=True,
                            stop=True,
                        ).then_inc(s_row[0], 1)
                e.wait_ge(s_sq[1], 1)
                e.matmul(
                    prow[1][:, :, :],
                    ones[:, :],
                    sq_all[:, 1, :, :],
                    start=True,
                    stop=True,
                ).then_inc(s_row[1], 1)

            @block.scalar
            def _(e):
                for i in range(NCHUNK):
                    e.wait_ge(s_tp, i + 1)
                    w = CHUNKS[i][2]
                    # fold the 1/64 spatial mean into the transpose copy (exact)
                    e.activation(
                        copy_dst(i), pt[i % 2][:w, :], ACTF.Copy, scale=1.0 / S
                    ).then_inc(s_cp, 1)
                    if i == NX - 1:
                        e.wait_ge(s_hmm, 1)
                        for m in range(4):
                            e.activation(
                                hT[:, m, :],
                                ph[:, m, :],
                                ACTF.Relu,
                                bias=wsb[:, 3072 + m:3073 + m],
                            ).then_inc(s_relu, 1)
                    if i == NX:
                        e.wait_ge(s_mumm, 1)
                        for m in range(2):
                            e.activation(
                                muT[:, m, :],
                                pmu[:, m, :],
                                ACTF.Identity,
                                bias=wsb[:, 3076 + m:3077 + m],
                            ).then_inc(s_mubias, 1)
                    if i == 23:
                        e.wait_ge(s_row[0], 1)
                        e.activation(
                            rows[:, 0, :, :], prow[0][:, :, :], ACTF.Copy
                        ).then_inc(s_rowcp[0], 1)
                e.wait_ge(s_row[1], 1)
                e.activation(
                    rows[:, 1, :, :], prow[1][:, :, :], ACTF.Copy
                ).then_inc(s_rowcp[1], 1)

    return nc


def _get_nc():
    if "nc" not in _CACHE:
        _CACHE["nc"] = build_nc()
    return _CACHE["nc"]


def make_in_maps(x_samples, y_samples, W1, b1, W2, b2):
    xs = np.ascontiguousarray(
        np.asarray(x_samples, np.float32).reshape(N, XC, S)
    )
    ys = np.ascontiguousarray(
        np.asarray(y_samples, np.float32).reshape(N, YC, S)
    )
    wp = np.zeros((P, 3078), np.float32)
    wp[:, :2048] = (
        np.asarray(W1, np.float32).reshape(4, P, HID).transpose(1, 0, 2).reshape(P, 2048)
    )
    wp[:, 2048:3072] = (
        np.asarray(W2, np.float32).reshape(4, P, YC).transpose(1, 0, 2).reshape(P, 1024)
    )
    wp[:, 3072:3076] = np.asarray(b1, np.float32).reshape(4, P).T
    wp[:, 3076:3078] = np.asarray(b2, np.float32).reshape(2, P).T
    wp = np.ascontiguousarray(wp)
    in_maps = []
    for c in range(8):
        in_maps.append(
            {
                "x": np.ascontiguousarray(xs[c * P:(c + 1) * P]),
                "y": np.ascontiguousarray(ys[c * P:(c + 1) * P]),
                "wpack": wp,
            }
        )
    return in_maps


def combine(results):
    A = B = S2 = 0.0
    EyN = np.zeros(YC, np.float64)
    MuN = np.zeros(YC, np.float64)
    for c in range(8):
        vec = results[c]["out_vec"].astype(np.float64)    # (128, 4)
        row = results[c]["out_row"].astype(np.float64)    # (1, 2, 3, 128)
        EyN += np.concatenate([vec[:, 0], vec[:, 1]])
        MuN += np.concatenate([vec[:, 2], vec[:, 3]])
        A += row[0, :, 0, :].sum()
        B += row[0, :, 1, :].sum()
        S2 += row[0, :, 2, :].sum()
    ey = EyN / N
    mu = MuN / N
    loss = -(A / N) / 2.0 + 0.5 * (S2 / N - 2.0 * float(mu @ ey) + B / N)
    return np.float32(loss)


def run(inputs, **kwargs):
    nc = _get_nc()
    in_maps = make_in_maps(**inputs)
    res = run_bass_kernel_spmd(nc, in_maps, core_ids=list(range(8)), **kwargs)
    return combine(res.results), res


def kernel(x_samples, y_samples, W1, b1, W2, b2):
    loss, _ = run(
        dict(
            x_samples=x_samples,
            y_samples=y_samples,
            W1=W1,
            b1=b1,
            W2=W2,
            b2=b2,
        )
    )
    return loss
